# revision 1
# baseline (speedup 1.0000x reference)
"""VQ codebook reconstruction kernel for Trainium2 (8 NeuronCores, SPMD).

Reference computation (per pixel feature vector f in R^C):
    weights = (codebook @ f) / ||codebook_rows||^2      # [N]
    recon   = codebook.T @ weights                      # [C]

This collapses to a single fixed matrix applied per pixel:
    recon = M @ f,   M = codebook.T @ diag(1/||c_n||^2) @ codebook   # [C, C]

M is tiny ([256,256]) and is formed on the host in float64; the device
kernel applies M to all B*H*W = 131072 pixel vectors, sharded
data-parallel over (B, H) across 8 cores. Matmuls use float32r
(fp32 with 11-bit mantissa) which streams at full PE rate (1
cycle/row) for moving dim >= 256, unlike plain fp32 (4 cycles/row).
The output is written as fp16 (RNE, exact host upcast to fp32) to
halve write traffic; total scale-relative error ~4.8e-4. The kernel
is read-bandwidth-bound in the front half (16.9 MB/core at ~420 GB/s)
and matmul+copy-pipeline-bound in the back half, at ~81 us measured,
plus the fixed ~7 us NEFF preamble and ~10 us exit barrier.
"""

import numpy as np

B, C, H, W = 4, 256, 128, 256
N_CORES = 8
SPLIT_H = 2            # 8 shards = B(4) x H-halves(2)
SH = H // SPLIT_H      # 64 rows of H per shard
P_SHARD = SH * W       # 16384 pixels per core
TILE_N = 512
N_TILES = P_SHARD // TILE_N  # 32

_NC_CACHE = {}


def _build_nc():
    if "nc" in _NC_CACHE:
        return _NC_CACHE["nc"]

    import concourse.bass as bass
    import concourse.tile as tile
    from concourse import bacc, mybir

    f32 = mybir.dt.float32
    f16 = mybir.dt.float16
    f32r = mybir.dt.float32r

    nc = bacc.Bacc()
    feat = nc.dram_tensor("feat", [C, P_SHARD], f32r, kind="ExternalInput")
    mmat = nc.dram_tensor("mmat", [C, C], f32r, kind="ExternalInput")
    # fp16 output halves write traffic; host upcasts to fp32 (exact).
    # Output magnitudes are O(10) — far inside fp16 range; quantization
    # adds ~4.9e-4 scale-relative error on top of f32r's 2.4e-4.
    out = nc.dram_tensor("out", [C, P_SHARD], f16, kind="ExternalOutput")

    SLAB = 2048
    N_SLABS = P_SHARD // SLAB          # 8
    SUB = SLAB // TILE_N               # 4 matmul subtiles per slab

    # feat rows are (kb*128 + p); view as [p, kb, n] so one DMA per slab
    # pulls both K-halves.
    feat3 = feat.rearrange("(a k) n -> k a n", a=2)

    with tile.TileContext(nc) as tc:
        with (
            tc.tile_pool(name="mpool", bufs=1) as mpool,
            tc.tile_pool(name="rhs", bufs=8) as rhs_pool,
            tc.tile_pool(name="opool", bufs=3) as opool,
            tc.tile_pool(name="psum", bufs=4, space="PSUM") as psum_pool,
        ):
            # M as two [128, 256] K-halves; lhsT block for (kb, mb) is
            # m_tiles[kb][:, mb*128:(mb+1)*128] (M is symmetric so lhsT = M).
            m_tiles = []
            for kb in range(2):
                mt = mpool.tile([128, C], f32r, tag=f"m{kb}")
                nc.gpsimd.dma_start(mt[:], mmat[kb * 128:(kb + 1) * 128, :])
                m_tiles.append(mt)

            for j in range(N_SLABS):
                rt = rhs_pool.tile([128, 2, SLAB], f32r, tag="r")
                eng = nc.sync if (j % 2 == 0) else nc.scalar
                eng.dma_start(rt[:], feat3[:, :, bass.ts(j, SLAB)])
                ot = [
                    opool.tile([128, SLAB], f16, tag=f"o{mb}", name=f"ot{mb}")
                    for mb in range(2)
                ]
                for n in range(SUB):
                    for mb in range(2):
                        ps = psum_pool.tile([128, TILE_N], f32, tag=f"ps{mb}")
                        for kb in range(2):
                            nc.tensor.matmul(
                                ps[:],
                                m_tiles[kb][:, mb * 128:(mb + 1) * 128],
                                rt[:, kb, bass.ts(n, TILE_N)],
                                start=(kb == 0),
                                stop=(kb == 1),
                            )
                        nc.vector.tensor_copy(ot[mb][:, bass.ts(n, TILE_N)], ps[:])
                for mb in range(2):
                    nc.gpsimd.dma_start(
                        out[mb * 128:(mb + 1) * 128, bass.ts(j, SLAB)], ot[mb][:]
                    )

    nc.compile()
    _NC_CACHE["nc"] = nc
    return nc


def _host_prep(feature, codebook):
    cb = codebook.astype(np.float64)
    norm = np.sum(cb * cb, axis=1)
    m = ((cb / norm[:, None]).T @ cb).astype(np.float32)

    in_maps = []
    for i in range(N_CORES):
        b, hs = i // SPLIT_H, (i % SPLIT_H) * SH
        shard = np.ascontiguousarray(
            feature[b, :, hs:hs + SH, :].reshape(C, P_SHARD)
        )
        in_maps.append({"feat": shard, "mmat": m})
    return in_maps


def _gather(results):
    out = np.empty((B, C, H, W), dtype=np.float32)
    for i in range(N_CORES):
        b, hs = i // SPLIT_H, (i % SPLIT_H) * SH
        out[b, :, hs:hs + SH, :] = results[i]["out"].reshape(C, SH, W).astype(np.float32)
    return out


def run(feature, codebook, **spmd_kwargs):
    from concourse.bass_utils import run_bass_kernel_spmd

    nc = _build_nc()
    in_maps = _host_prep(np.asarray(feature), np.asarray(codebook))
    res = run_bass_kernel_spmd(nc, in_maps, list(range(N_CORES)), **spmd_kwargs)
    return _gather(res.results), res


def kernel(feature, codebook):
    out, _ = run(feature, codebook)
    return out



# revision 2
# speedup vs baseline: 1.2415x; 1.2415x over previous
"""VQ codebook reconstruction kernel for Trainium2 (8 NeuronCores, SPMD).

Reference computation (per pixel feature vector f in R^C):
    weights = (codebook @ f) / ||codebook_rows||^2      # [N]
    recon   = codebook.T @ weights                      # [C]

This collapses to a single fixed matrix applied per pixel:
    recon = M @ f,   M = codebook.T @ diag(1/||c_n||^2) @ codebook   # [C, C]

M is tiny ([256,256]) and is formed on the host in float64; the device
kernel applies M to all B*H*W = 131072 pixel vectors, sharded
data-parallel over (B, H) across 8 cores.

v2 changes vs the 81 us baseline:
  - feature and M are cast to bf16 on the host: halves the read traffic
    (8.39 MB/core) and keeps the PE at 1 col/cycle (same as f32r).
    Error ~2^-9 per element, well inside the 2e-2 gate.
  - all DMA goes through the two HWDGE rings (sync = reads, scalar =
    writes); the baseline wrote through gpsimd SWDGE at 131 GB/s with a
    multi-us descriptor/receipt tail.
  - PSUM->SBUF copies are split between the vector (DVE) and scalar
    (ACT) engines instead of serializing ~34 us on DVE alone.
  - both 128-row output halves are packed in one [128, 2, SLAB] tile so
    each slab is a single 1 MB write DMA.
"""

import numpy as np
import ml_dtypes

B, C, H, W = 4, 256, 128, 256
N_CORES = 8
SPLIT_H = 2            # 8 shards = B(4) x H-halves(2)
SH = H // SPLIT_H      # 64 rows of H per shard
P_SHARD = SH * W       # 16384 pixels per core
TILE_N = 512
SLAB = 2048
N_SLABS = P_SHARD // SLAB          # 8
SUB = SLAB // TILE_N               # 4 matmul subtiles per slab

_NC_CACHE = {}


def _build_nc():
    if "nc" in _NC_CACHE:
        return _NC_CACHE["nc"]

    import concourse.bass as bass
    import concourse.tile as tile
    from concourse import bacc, mybir

    f32 = mybir.dt.float32
    f16 = mybir.dt.float16
    bf16 = mybir.dt.bfloat16

    nc = bacc.Bacc()
    feat = nc.dram_tensor("feat", [C, P_SHARD], bf16, kind="ExternalInput")
    mmat = nc.dram_tensor("mmat", [C, C], bf16, kind="ExternalInput")
    # out[p, mb, n] = recon[mb*128 + p, n]; packing both row-halves lets
    # one DMA per slab carry the full 1 MB. fp16 halves write traffic;
    # host upcasts to fp32 (exact).
    out = nc.dram_tensor("out", [128, 2, P_SHARD], f16, kind="ExternalOutput")

    # feat rows are (kb*128 + p); view as [p, kb, n] so one DMA per slab
    # pulls both K-halves.
    feat3 = feat.rearrange("(a k) n -> k a n", a=2)

    with tile.TileContext(nc) as tc:
        with (
            tc.tile_pool(name="mpool", bufs=1) as mpool,
            tc.tile_pool(name="rhs", bufs=6) as rhs_pool,
            tc.tile_pool(name="opool", bufs=3) as opool,
            tc.tile_pool(name="psum", bufs=4, space="PSUM") as psum_pool,
        ):
            # M as two [128, 256] K-halves; lhsT block for (kb, mb) is
            # m_tiles[kb][:, mb*128:(mb+1)*128] (M is symmetric so lhsT = M).
            m_tiles = []
            for kb in range(2):
                mt = mpool.tile([128, C], bf16, tag=f"m{kb}")
                nc.sync.dma_start(mt[:], mmat[kb * 128:(kb + 1) * 128, :])
                m_tiles.append(mt)

            for j in range(N_SLABS):
                rt = rhs_pool.tile([128, 2, SLAB], bf16, tag="r")
                nc.sync.dma_start(rt[:], feat3[:, :, bass.ts(j, SLAB)])
                ot = opool.tile([128, 2, SLAB], f16, tag="o", name="ot")
                for n in range(SUB):
                    for mb in range(2):
                        ps = psum_pool.tile([128, TILE_N], f32, tag=f"ps{mb}")
                        for kb in range(2):
                            nc.tensor.matmul(
                                ps[:],
                                m_tiles[kb][:, mb * 128:(mb + 1) * 128],
                                rt[:, kb, bass.ts(n, TILE_N)],
                                start=(kb == 0),
                                stop=(kb == 1),
                            )
                        # DVE takes the first half of each slab, ACT the
                        # second, so ACT's own last copy is the final
                        # dependency of the write DMA it triggers.
                        if n < 2:
                            nc.vector.tensor_copy(ot[:, mb, bass.ts(n, TILE_N)], ps[:])
                        else:
                            nc.scalar.copy(ot[:, mb, bass.ts(n, TILE_N)], ps[:])
                nc.scalar.dma_start(out[:, :, bass.ts(j, SLAB)], ot[:])

    nc.compile()
    _NC_CACHE["nc"] = nc
    return nc


def _host_prep(feature, codebook):
    cb = codebook.astype(np.float64)
    norm = np.sum(cb * cb, axis=1)
    m = ((cb / norm[:, None]).T @ cb).astype(ml_dtypes.bfloat16)

    feature = np.asarray(feature)
    in_maps = []
    for i in range(N_CORES):
        b, hs = i // SPLIT_H, (i % SPLIT_H) * SH
        shard = np.ascontiguousarray(
            feature[b, :, hs:hs + SH, :].reshape(C, P_SHARD)
        ).astype(ml_dtypes.bfloat16)
        in_maps.append({"feat": shard, "mmat": m})
    return in_maps


def _gather(results):
    out = np.empty((B, C, H, W), dtype=np.float32)
    for i in range(N_CORES):
        b, hs = i // SPLIT_H, (i % SPLIT_H) * SH
        r = results[i]["out"]  # [128, 2, P_SHARD] fp16
        out[b, :, hs:hs + SH, :] = (
            r.transpose(1, 0, 2).reshape(C, SH, W).astype(np.float32)
        )
    return out


def run(feature, codebook, **spmd_kwargs):
    from concourse.bass_utils import run_bass_kernel_spmd

    nc = _build_nc()
    in_maps = _host_prep(np.asarray(feature), np.asarray(codebook))
    res = run_bass_kernel_spmd(nc, in_maps, list(range(N_CORES)), **spmd_kwargs)
    return _gather(res.results), res


def kernel(feature, codebook):
    out, _ = run(feature, codebook)
    return out
